# revision 1
# baseline (speedup 1.0000x reference)
"""AfmoeTokenChoiceRouter kernel for 8x Trainium2 NeuronCores.

Data-parallel over tokens: each of the 8 cores handles 2048 tokens (16.8 MB
of activations -> ~47 us HBM roofline per core at 358 GB/s).

Precision scheme: x and gate_w are split on the host into fp16 hi + fp16 lo
pairs (x = xh + xl exactly up to ~2^-22 relative). The device computes the
full four-term product (xh+xl)@(wh+wl)^T with fp16 operands and fp32 PSUM
accumulation, giving fp32-class logits (top-8 selection matches a pure-fp32
reference on all but ~1 near-tie token in 16384) at bf16 PE throughput.

Per core pipeline (4 supertiles of 512 tokens, double/triple buffered):
  - DMA: xh/xl tiles in transposed [H, token] layout (host pre-packed to the
    exact SBUF layout, so every DMA is a contiguous 1 MiB burst)
  - PE: per k-chunk one [wh_c | wl_c] packed 128-wide stationary, 2 matmuls
    (rhs=xh_c, rhs=xl_c): psum rows 0:64 accumulate the wh terms, rows
    64:128 the wl terms -> 32 matmuls per supertile
  - PE: full [128,128] back-transposes -> [token, 2*64] layout
  - DVE adds the two 64-column halves (the hi/lo combine), ACT sigmoid
  - DVE top-8: max8/max_index on biased scores (exact fp32 selection),
    threshold mask + second max8 pass on masked unbiased scores, 8x8
    index-match to reorder into biased-rank order, normalize, scale by 2.5
Outputs per core: scores [128, 16, 8] f32 and indices [128, 16, 8] u32 in
partition-major token order (token = 128*tile + partition), unpermuted on
the host.
"""

import numpy as np

import concourse.bass as bass
import concourse.mybir as mybir
import concourse.tile as tile
import concourse.bass_utils as bass_utils
from concourse import bacc
from concourse.masks import make_identity

f32 = mybir.dt.float32
f16 = mybir.dt.float16
u32 = mybir.dt.uint32
Alu = mybir.AluOpType
Act = mybir.ActivationFunctionType

N_CORES = 8
T_FULL, H, E, TOPK = 16384, 2048, 64, 8
T_CORE = T_FULL // N_CORES          # 2048
TOK_ST = 512                        # tokens per supertile
N_ST = T_CORE // TOK_ST             # 4
TILES_ST = TOK_ST // 128            # 4
N_TILES = T_CORE // 128             # 16
N_CH = H // 128                     # 16 contraction chunks
ROUTE_SCALE = 2.5


def router_body(tc, outs, ins, reps=1, skip_dma=False, skip_compute=False, n_terms=3, skip_topk=False):
    """Emit the per-core program. outs = (scores[128, N_TILES*8] f32,
    idx[128, N_TILES*8] u32); ins = (xh[N_ST,128,N_CH*TOK_ST] f16,
    xl same, w2[128,N_CH*128] f16 (wh|wl packed), bias[128,E] f32)."""
    nc = tc.nc
    out_s_d, out_i_d = outs
    xh_d, xl_d, w2_d, bias_d = ins

    with (
        tc.tile_pool(name="const", bufs=1) as constp,
        tc.tile_pool(name="xin", bufs=globals().get('_XBUFS', 3)) as xpool,
        tc.tile_pool(name="persist", bufs=1) as pers,
        tc.tile_pool(name="scratch", bufs=globals().get('_SCRBUFS', 3)) as scr,
        tc.tile_pool(name="ps_lt", bufs=globals().get('_LTBUFS', 3), space="PSUM") as ps_lt,
        tc.tile_pool(name="ps_l", bufs=globals().get('_PLBUFS', 3), space="PSUM") as ps_l,
    ):
        ident = constp.tile([128, 128], f32)
        make_identity(nc, ident[:])
        # setup DMAs ride the ACT HWDGE queue so they don't delay the first
        # x pieces on the sync queue (HWDGE is FIFO per issuing engine)
        w2_sb = constp.tile([128, N_CH, 128], f16)
        nc.scalar.dma_start(w2_sb[:], w2_d)
        bias_sb = constp.tile([128, 1, E], f32)
        nc.scalar.dma_start(bias_sb[:], bias_d)

        # persistent per-core tensors
        s_all = pers.tile([128, N_TILES, E], f32)      # sigmoid scores
        b_all = pers.tile([128, N_TILES, E], f32)      # biased scores
        vb_all = pers.tile([128, N_TILES, 8], f32)     # top8 of biased
        vs_all = pers.tile([128, N_TILES, 8], f32)     # top8 of masked s
        ib_all = pers.tile([128, N_TILES, 8], u32)     # indices (biased order)
        is_all = pers.tile([128, N_TILES, 8], u32)     # indices (s order)
        ibf = pers.tile([128, N_TILES, 8], f32)
        isf = pers.tile([128, N_TILES, 8], f32)
        out_s_sb = pers.tile([128, N_TILES, 8], f32)

        DMA_CH = globals().get('_DMA_CH_OVERRIDE', 8)  # h-chunks per DMA piece

        def supertile(pos, tok_st):
            tiles_ss = tok_st // 128
            g, off = pos // TOK_ST, pos % TOK_ST
            t0 = pos // 128
            s4 = slice(t0, t0 + tiles_ss)
            xh_sb = xpool.tile([128, N_CH, tok_st], f16, tag="xh")
            xl_sb = xpool.tile([128, N_CH, tok_st], f16, tag="xl")
            tsl = slice(off, off + tok_st)
            xh_st = xh_d[g].rearrange("p (c t) -> p c t", t=TOK_ST)[:, :, tsl]
            xl_st = xl_d[g].rearrange("p (c t) -> p c t", t=TOK_ST)[:, :, tsl]
            if not skip_dma:
                xl_eng = nc.scalar if globals().get('_XL_ON_ACT', 1) else nc.sync
                pieces = globals().get('_PIECES', None)
                if pieces is None:
                    pieces = []
                    d0 = 0
                    while d0 < N_CH:
                        pieces.append((d0, min(DMA_CH, N_CH - d0)))
                        d0 += min(DMA_CH, N_CH - d0)
                for d0, dn in pieces:
                    dsl = slice(d0, d0 + dn)
                    nc.sync.dma_start(xh_sb[:, dsl, :], xh_st[:, dsl, :])
                    xl_eng.dma_start(xl_sb[:, dsl, :], xl_st[:, dsl, :])
            if skip_compute:
                return

            # GEMM: stationary [wh_c | wl_c] packed as one [128, 128] weight.
            # psum rows 0:64 accumulate wh terms, rows 64:128 wl terms;
            # both xh and xl stream against the same stationary.
            lt_ps = ps_lt.tile([128, tok_st], f32, tag="lt")
            for c in range(N_CH):
                w2_c = w2_sb[:, c, :]
                last = c == N_CH - 1
                nc.tensor.matmul(lt_ps[:], w2_c, xh_sb[:, c, :],
                                 start=(c == 0), stop=(last and n_terms == 1))
                if n_terms >= 3:
                    nc.tensor.matmul(lt_ps[:], w2_c, xl_sb[:, c, :],
                                     start=False, stop=last)
            if n_terms < 3:
                dummy = scr.tile([128, 1], f16, tag="dummy")
                nc.vector.tensor_copy(dummy[:], xl_sb[:, 0, :1])

            lt_sb = scr.tile([128, tok_st], f32, tag="ltsb")
            nc.scalar.copy(lt_sb[:], lt_ps[:])

            # full back-transpose per 128-token tile:
            # psum_l[:, q, 0:64] = wh-half logitsT.T, [:, q, 64:128] = wl-half
            l_ps = ps_l.tile([128, tiles_ss, 128], f32, tag="lps")
            for q in range(tiles_ss):
                nc.tensor.transpose(
                    l_ps[:, q, :],
                    lt_sb[:, q * 128:(q + 1) * 128],
                    ident[:],
                )

            # combine halves: logits[tok, e] = hi + lo (lanes aligned; DVE can
            # read only one PSUM operand, so stage the lo half through SBUF,
            # then add it back into the hi half in place -- sigmoid reads PSUM)
            half_sb = scr.tile([128, tiles_ss, E], f32, tag="half")
            nc.scalar.copy(half_sb[:], l_ps[:, :, E:2 * E])
            nc.vector.tensor_tensor(out=l_ps[:, :, 0:E], in0=l_ps[:, :, 0:E],
                                    in1=half_sb[:], op=Alu.add)

            s_sl = s_all[:, s4, :]
            nc.scalar.activation(s_sl, l_ps[:, :, 0:E], Act.Sigmoid)
            if skip_topk:
                nc.vector.tensor_copy(out_s_sb[:, s4, :], s_sl[:, :, :8])
                nc.vector.tensor_copy(ib_all[:, s4, :], s_sl[:, :, 8:16])
                return
            b_sl = b_all[:, s4, :]
            nc.vector.tensor_tensor(
                out=b_sl, in0=s_sl,
                in1=bias_sb[:].broadcast_to([128, tiles_ss, E]),
                op=Alu.add,
            )

            for q in range(tiles_ss):
                i = t0 + q
                nc.vector.max(out=vb_all[:, i, :], in_=b_all[:, i, :])
                nc.vector.max_index(out=ib_all[:, i, :], in_max=vb_all[:, i, :],
                                    in_values=b_all[:, i, :])

            # selected-expert masking: sarr = (b >= thr8) * s
            variant = globals().get('_TOPK_VARIANT', 0)
            sarr = scr.tile([128, tiles_ss, E], f32, tag="sarr")
            if variant in (1, 3):
                for q in range(tiles_ss):
                    i = t0 + q
                    nc.vector.scalar_tensor_tensor(
                        out=sarr[:, q, :], in0=b_all[:, i, :],
                        scalar=vb_all[:, i, 7:8], in1=s_all[:, i, :],
                        op0=Alu.is_ge, op1=Alu.mult)
            else:
                eng = nc.gpsimd if variant == 2 else nc.vector
                thr = vb_all[:, s4, 7:8].broadcast_to([128, tiles_ss, E])
                mask = scr.tile([128, tiles_ss, E], f32, tag="mask")
                eng.tensor_tensor(out=mask[:], in0=b_sl, in1=thr, op=Alu.is_ge)
                eng.tensor_tensor(out=sarr[:], in0=s_sl, in1=mask[:], op=Alu.mult)

            for q in range(tiles_ss):
                i = t0 + q
                nc.vector.max(out=vs_all[:, i, :], in_=sarr[:, q, :])
                nc.vector.max_index(out=is_all[:, i, :], in_max=vs_all[:, i, :],
                                    in_values=sarr[:, q, :])

            # reorder vs_all (s-descending) into biased-rank order by idx match
            nc.vector.tensor_copy(ibf[:, s4, :], ib_all[:, s4, :])
            nc.vector.tensor_copy(isf[:, s4, :], is_all[:, s4, :])
            eng2 = nc.gpsimd if variant in (2, 3) else nc.vector
            eq = scr.tile([128, tiles_ss, 8, 8], f32, tag="eq")
            eng2.tensor_tensor(
                out=eq[:],
                in0=ibf[:, s4, :].broadcast_to([128, tiles_ss, 8, 8]),
                in1=isf[:, s4, :][:, :, None, :].broadcast_to(
                    [128, tiles_ss, 8, 8]),
                op=Alu.is_equal,
            )
            g_sc = scr.tile([128, tiles_ss, 8, 8], f32, tag="g")
            eng2.tensor_tensor(
                out=g_sc[:], in0=eq[:],
                in1=vs_all[:, s4, :][:, :, None, :].broadcast_to(
                    [128, tiles_ss, 8, 8]),
                op=Alu.mult,
            )
            tsr = scr.tile([128, tiles_ss, 8], f32, tag="tsr")
            nc.vector.reduce_sum(out=tsr[:], in_=g_sc[:], axis=mybir.AxisListType.X)

            den = scr.tile([128, tiles_ss], f32, tag="den")
            nc.vector.reduce_sum(out=den[:], in_=vs_all[:, s4, :],
                                 axis=mybir.AxisListType.X)
            rec = scr.tile([128, tiles_ss], f32, tag="rec")
            nc.vector.reciprocal(rec[:], den[:])
            nc.vector.scalar_tensor_tensor(
                out=out_s_sb[:, s4, :], in0=tsr[:], scalar=ROUTE_SCALE,
                in1=rec[:].broadcast_to([128, tiles_ss, 8]),
                op0=Alu.mult, op1=Alu.mult,
            )
            od_s = out_s_d.rearrange("p (i k) -> p i k", k=8)
            od_i = out_i_d.rearrange("p (i k) -> p i k", k=8)
            nc.gpsimd.dma_start(od_s[:, s4, :], out_s_sb[:, s4, :])
            nc.gpsimd.dma_start(od_i[:, s4, :], ib_all[:, s4, :])

        # last supertiles shrink so the final serial DVE top-k tail is short
        schedule = globals().get('_SCHEDULE_OVERRIDE', [512, 512, 512, 384, 128])
        assert sum(schedule) == T_CORE

        def whole_pass():
            pos = 0
            for tok_st in schedule:
                supertile(pos, tok_st)
                pos += tok_st
            if skip_compute:
                return

        if reps == 1:
            whole_pass()
        else:
            with tc.For_i(0, reps, 1):
                whole_pass()


def build_nc(reps=1, skip_dma=False, skip_compute=False, n_terms=3, skip_topk=False):
    nc = bacc.Bacc("TRN2", target_bir_lowering=False, debug=False)
    xh_d = nc.dram_tensor("xh_d", [N_ST, 128, N_CH * TOK_ST], f16, kind="ExternalInput")
    xl_d = nc.dram_tensor("xl_d", [N_ST, 128, N_CH * TOK_ST], f16, kind="ExternalInput")
    w2_d = nc.dram_tensor("w2_d", [128, N_CH * 128], f16, kind="ExternalInput")
    bias_d = nc.dram_tensor("bias_d", [128, E], f32, kind="ExternalInput")
    out_s_d = nc.dram_tensor("out_s_d", [128, N_TILES * 8], f32, kind="ExternalOutput")
    out_i_d = nc.dram_tensor("out_i_d", [128, N_TILES * 8], u32, kind="ExternalOutput")

    with tile.TileContext(nc) as tc:
        router_body(
            tc,
            (out_s_d.ap(), out_i_d.ap()),
            (xh_d.ap(), xl_d.ap(), w2_d.ap(), bias_d.ap()),
            reps=reps, skip_dma=skip_dma, skip_compute=skip_compute,
            n_terms=n_terms, skip_topk=skip_topk,
        )
    nc.compile()
    return nc


def pack_x_shard(xh_shard_T):
    """[H, T_CORE] fp16 -> [N_ST, 128, N_CH*TOK_ST] in SBUF tile layout:
    out[st, p, c*TOK_ST + t] = xT[c*128 + p, st*TOK_ST + t]."""
    v = xh_shard_T.reshape(N_CH, 128, N_ST, TOK_ST)
    return np.ascontiguousarray(v.transpose(2, 1, 0, 3)).reshape(N_ST, 128, N_CH * TOK_ST)


def pack_w2(wh, wl):
    """wh/wl [E, H] fp16 -> [128, N_CH*128] with wh in cols 0:64, wl in 64:128
    of each chunk: out[p, c*128 + e] = (wh if e < E else wl)[e % E, c*128 + p]."""
    vh = wh.T.reshape(N_CH, 128, E)
    vl = wl.T.reshape(N_CH, 128, E)
    v = np.concatenate([vh, vl], axis=2)          # [N_CH, 128, 128]
    return np.ascontiguousarray(v.transpose(1, 0, 2)).reshape(128, N_CH * 128)


_NC_CACHE = {}


def kernel(hidden_states, expert_bias, gate_w):
    x2 = np.asarray(hidden_states, dtype=np.float32).reshape(T_FULL, H)
    w = np.asarray(gate_w, dtype=np.float32)
    bias = np.asarray(expert_bias, dtype=np.float32)

    xh = x2.astype(np.float16)
    xl = (x2 - xh.astype(np.float32)).astype(np.float16)
    wh = w.astype(np.float16)
    wl = (w - wh.astype(np.float32)).astype(np.float16)

    w2_p = pack_w2(wh, wl)
    bias_p = np.ascontiguousarray(np.broadcast_to(bias[None, :], (128, E)))

    in_maps = []
    for k in range(N_CORES):
        rows = slice(k * T_CORE, (k + 1) * T_CORE)
        in_maps.append({
            "xh_d": pack_x_shard(np.ascontiguousarray(xh[rows].T)),
            "xl_d": pack_x_shard(np.ascontiguousarray(xl[rows].T)),
            "w2_d": w2_p,
            "bias_d": bias_p,
        })

    if "nc" not in _NC_CACHE:
        _NC_CACHE["nc"] = build_nc()
    nc = _NC_CACHE["nc"]

    res = bass_utils.run_bass_kernel_spmd(nc, in_maps, core_ids=list(range(N_CORES)))

    scores = np.empty((T_FULL, TOPK), dtype=np.float32)
    idx = np.empty((T_FULL, TOPK), dtype=np.int32)
    for k in range(N_CORES):
        o = res.results[k]
        s = o["out_s_d"].reshape(128, N_TILES, TOPK).transpose(1, 0, 2).reshape(T_CORE, TOPK)
        i = o["out_i_d"].view(np.int32).reshape(128, N_TILES, TOPK).transpose(1, 0, 2).reshape(T_CORE, TOPK)
        scores[k * T_CORE:(k + 1) * T_CORE] = s
        idx[k * T_CORE:(k + 1) * T_CORE] = i
    return scores, idx



# revision 39
# speedup vs baseline: 5.9154x; 5.9154x over previous
"""AfmoeTokenChoiceRouter kernel for 8x Trainium2 NeuronCores.

Data-parallel over tokens: each of the 8 cores handles 2048 tokens.

Precision scheme (3 bytes/element of x instead of 4):
  x  = xh (fp16) + r,   r shipped as xl8 = e4m3(r * 2^16)     [1 byte]
  w  = wh (fp16) + wl (fp16)  [replicated, tiny]
  w8 = e4m3(w * 2^11)   [replicated, tiny]
The fp16 stream is shipped pre-scaled (xh*2^13, w*2^14 -- exact exponent
shifts) so its products land at x*w*2^27, the SAME scale as the fp8 stream's
(r*2^16)*(w*2^11): both streams accumulate into ONE PSUM region and the
2^-27 folds into the combine constant. Top-8 selection matches a pure-fp32
reference on all but ~4 near-tie tokens in 16384 (L2 idx rel err ~5e-3),
while HBM traffic drops from 16.8 MB to 12.6 MB per core.

Per core pipeline (supertile blocks per SCHEDULE, host-packed so every DMA
is a contiguous full-rate burst; tapering tail keeps the serial drain short):
  - DMA: xh (fp16) on the sync HWDGE queue, xl8 (fp8) on the ACT queue
  - PE per block: 16 fp16 matmuls ([wh_c|wl_c] stationary x xh_c -> psum
    rows 0:64 wh terms + 2^-13-aligned, 64:128 wl terms), then 8 fp8
    DoubleRow matmuls (2 k-chunks each, w8 pairs stationary) accumulating
    the residual into rows 0:64 of the same psum. Grouping the fp8 stream
    after the fp16 stream (one dtype/stationary switch per block) measured
    ~1.6x faster than interleaving them per-chunk on hardware.
  - PE per 128-token tile: one "J-matmul" (data block stationary, constant
    jA = 2^-27*[I64;I64] moving) fuses the back-transpose, the hi+lo row
    fold and the 2^-27 descale in a single instruction -> l_ps [tok, 64]
  - ACT sigmoid; DVE top-8: max8/max_index on biased scores, threshold
    mask + second max8 pass on masked unbiased scores, 8x8 index-match
    reorder, normalize, scale by 2.5
  - outputs stored via the ACT HWDGE queue (head tiles early, short tail
    at the end); SWDGE descriptor generation (~6us per strided store) and
    per-block store dribble both measured as tail serializers
Outputs per core: scores [128, 16, 8] f32 and indices [128, 16, 8] u32 in
partition-major token order (token = 128*tile + partition), unpermuted on
the host.
"""

import numpy as np
import ml_dtypes

import concourse.bass as bass
import concourse.mybir as mybir
import concourse.tile as tile
import concourse.bass_utils as bass_utils
from concourse import bacc
from concourse.masks import make_identity

f32 = mybir.dt.float32
f16 = mybir.dt.float16
f8 = mybir.dt.float8e4
u32 = mybir.dt.uint32
Alu = mybir.AluOpType
Act = mybir.ActivationFunctionType

N_CORES = 8
T_FULL, H, E, TOPK = 16384, 2048, 64, 8
T_CORE = T_FULL // N_CORES          # 2048
N_TILES = T_CORE // 128             # 16
N_CH = H // 128                     # 16 contraction chunks
ROUTE_SCALE = 2.5
# supertile schedule; baked into the host packing (each block is stored
# contiguously so every DMA is a full-rate linear burst). Tapering tail keeps
# the post-last-DMA serial chain short.
SCHEDULE = [384, 384, 384, 384, 256, 128, 128]
XA = 16                             # xl8 = e4m3(r * 2^XA), max |val| = 128
WB = 11                             # w8 = e4m3(w * 2^WB), max |val| = 224
# the fp16 stream is shipped pre-scaled (xh*2^13, w*2^14) so its PSUM terms
# land at x*w*2^27 == the fp8 stream's r*2^16 * w*2^11 scale: both streams
# accumulate into ONE psum, and the 2^-27 folds into the combine constant.
XS, WS = 13, 14
CSCALE = 2.0 ** (-(XA + WB))
assert XS + WS == XA + WB


def router_body(tc, outs, ins, reps=1, skip_dma=False, skip_compute=False,
                n_terms=3, skip_topk=False):
    nc = tc.nc
    out_s_d, out_i_d = outs
    xh_d, xl_d, w2_d, w8_d, bias_d, jab_d = ins

    with (
        tc.tile_pool(name="const", bufs=1) as constp,
        tc.tile_pool(name="xin", bufs=globals().get('_XBUFS', 3)) as xpool,
        tc.tile_pool(name="persist", bufs=1) as pers,
        tc.tile_pool(name="scratch", bufs=globals().get('_SCRBUFS', 3)) as scr,
        tc.tile_pool(name="ps_lt", bufs=globals().get('_LTBUFS', 3), space="PSUM") as ps_lt,
        tc.tile_pool(name="ps_l", bufs=globals().get('_PLBUFS', 3), space="PSUM") as ps_l,
    ):
        # setup DMAs ride the ACT HWDGE queue so they don't delay the first
        # x pieces on the sync queue (HWDGE is FIFO per issuing engine).
        # w2 is split into pieces so the first chunks' matmuls can start
        # before the whole 512 KB stationary lands; bias/jab (needed only
        # ~8us in) are deferred until after the first xl block (see below).
        w2_sb = constp.tile([128, N_CH, 128], f16)
        w8_sb = constp.tile([128, N_CH, E], f8)
        for d0 in range(0, N_CH, 4):
            nc.scalar.dma_start(w2_sb[:, d0:d0 + 4, :], w2_d.rearrange(
                "p (c e) -> p c e", e=128)[:, d0:d0 + 4, :])
        nc.scalar.dma_start(w8_sb[:], w8_d)
        bias_sb = constp.tile([128, 1, E], f32)
        # combine constants: jA = [I64; I64], jB = 2^-27 * I64. Used as the
        # moving operand of per-tile "transpose" matmuls that fuse the
        # back-transpose with the hi+lo add and the scaled fp8-term add.
        jab_sb = constp.tile([128, 1, E], f32)
        setup_rest = [(bias_sb, bias_d), (jab_sb, jab_d)]

        # persistent per-core tensors
        s_all = pers.tile([128, N_TILES, E], f32)      # sigmoid scores
        b_all = pers.tile([128, N_TILES, E], f32)      # biased scores
        vb_all = pers.tile([128, N_TILES, 8], f32)     # top8 of biased
        vs_all = pers.tile([128, N_TILES, 8], f32)     # top8 of masked s
        ib_all = pers.tile([128, N_TILES, 8], u32)     # indices (biased order)
        is_all = pers.tile([128, N_TILES, 8], u32)     # indices (s order)
        ibf = pers.tile([128, N_TILES, 8], f32)
        isf = pers.tile([128, N_TILES, 8], f32)
        out_s_sb = pers.tile([128, N_TILES, 8], f32)

        DMA_CH = globals().get('_DMA_CH_OVERRIDE', 4)   # h-chunks per xh piece
        DMA_CHL = globals().get('_DMA_CHL_OVERRIDE', 16)  # h-chunks per xl piece

        def supertile(pos, tok_st):
            tiles_ss = tok_st // 128
            t0 = pos // 128
            s4 = slice(t0, t0 + tiles_ss)
            xh_sb = xpool.tile([128, N_CH, tok_st], f16, tag="xh")
            xl_sb = xpool.tile([128, N_CH, tok_st], f8, tag="xl")
            foff = N_CH * pos
            xh_st = xh_d[:, foff:foff + N_CH * tok_st].rearrange(
                "p (c t) -> p c t", t=tok_st)
            xl_st = xl_d[:, foff:foff + N_CH * tok_st].rearrange(
                "p (c t) -> p c t", t=tok_st)
            if not skip_dma:
                xl_eng = nc.scalar if globals().get('_XL_ON_ACT', 1) else nc.sync
                d0 = 0
                while d0 < N_CH:
                    dn = min(DMA_CH, N_CH - d0)
                    nc.sync.dma_start(xh_sb[:, d0:d0 + dn, :], xh_st[:, d0:d0 + dn, :])
                    d0 += dn
                d0 = 0
                while d0 < N_CH:
                    dn = min(DMA_CHL, N_CH - d0)
                    xl_eng.dma_start(xl_sb[:, d0:d0 + dn, :], xl_st[:, d0:d0 + dn, :])
                    d0 += dn
                while setup_rest:
                    sb, dr = setup_rest.pop(0)
                    nc.scalar.dma_start(sb[:], dr)
            if skip_compute:
                return

            # GEMM: psumA <- [wh_c|wl_c] fp16 x xh_c (rows 0:64 wh, 64:128 wl)
            #       psumB <- w8_c fp8 x xl8_c (rows 0:64)
            # both streams accumulate into one psum: fp16 terms into rows
            # 0:128 ([wh|wl] stationary), fp8 residual terms into rows 0:64
            # (same scale 2^27 by construction)
            lt_ps = ps_lt.tile([128, tok_st], f32, tag="lt")
            use_dr = globals().get('_DR', 1)
            b_sep = globals().get('_BSEP', 1)
            for c in range(N_CH):
                last_a = c == N_CH - 1 and n_terms < 3
                nc.tensor.matmul(lt_ps[:], w2_sb[:, c, :], xh_sb[:, c, :],
                                 start=(c == 0), stop=last_a)
                if n_terms >= 3 and not use_dr and not b_sep:
                    nc.tensor.matmul(lt_ps[0:64, :], w8_sb[:, c, :],
                                     xl_sb[:, c, :],
                                     start=False, stop=(c == N_CH - 1))
            if n_terms >= 3 and not use_dr and b_sep:
                for c in range(N_CH):
                    nc.tensor.matmul(lt_ps[0:64, :], w8_sb[:, c, :],
                                     xl_sb[:, c, :],
                                     start=False, stop=(c == N_CH - 1))
            if n_terms >= 3 and use_dr:
                # fp8 DoubleRow: 2 contraction chunks per matmul
                # (out = sum_j lhsT[:, j, :].T @ rhs[:, j, :])
                for cc in range(N_CH // 2):
                    nc.tensor.matmul(
                        lt_ps[0:64, :], w8_sb[:, 2 * cc:2 * cc + 2, :],
                        xl_sb[:, 2 * cc:2 * cc + 2, :],
                        start=False, stop=(cc == N_CH // 2 - 1),
                        perf_mode=mybir.MatmulPerfMode.DoubleRow)
            if n_terms < 3:
                dummy = scr.tile([128, 1], f8, tag="dummy")
                nc.vector.tensor_copy(dummy[:], xl_sb[:, 0, :1])

            lt_sb = scr.tile([128, tok_st], f32, tag="ltsb")
            nc.scalar.copy(lt_sb[:], lt_ps[:])

            # fused back-transpose + combine: per 128-token block,
            #   l_ps[t, e] = sum_r lt[r, t] * jA[r, e] = 2^-27 * (hi + lo rows)
            # (the data block is the stationary, jA the 64-col moving)
            l_ps = ps_l.tile([128, tiles_ss, E], f32, tag="lps")
            for q in range(tiles_ss):
                qs = slice(q * 128, (q + 1) * 128)
                nc.tensor.matmul(l_ps[:, q, :], lt_sb[:, qs], jab_sb[:, 0, :],
                                 start=True, stop=True)

            s_sl = s_all[:, s4, :]
            nc.scalar.activation(s_sl, l_ps[:, :, :], Act.Sigmoid)
            if skip_topk:
                nc.vector.tensor_copy(out_s_sb[:, s4, :], s_sl[:, :, :8])
                nc.vector.tensor_copy(ib_all[:, s4, :], s_sl[:, :, 8:16])
                return
            b_sl = b_all[:, s4, :]
            beng = nc.gpsimd if globals().get('_BIAS_ON_POOL', 0) else nc.vector
            beng.tensor_tensor(
                out=b_sl, in0=s_sl,
                in1=bias_sb[:].broadcast_to([128, tiles_ss, E]),
                op=Alu.add,
            )

            for q in range(tiles_ss):
                i = t0 + q
                nc.vector.max(out=vb_all[:, i, :], in_=b_all[:, i, :])
                nc.vector.max_index(out=ib_all[:, i, :], in_max=vb_all[:, i, :],
                                    in_values=b_all[:, i, :])

            # selected-expert masking: sarr = (b >= thr8) * s
            sarr = scr.tile([128, tiles_ss, E], f32, tag="sarr")
            for q in range(tiles_ss):
                i = t0 + q
                nc.vector.scalar_tensor_tensor(
                    out=sarr[:, q, :], in0=b_all[:, i, :],
                    scalar=vb_all[:, i, 7:8], in1=s_all[:, i, :],
                    op0=Alu.is_ge, op1=Alu.mult)

            for q in range(tiles_ss):
                i = t0 + q
                nc.vector.max(out=vs_all[:, i, :], in_=sarr[:, q, :])
                nc.vector.max_index(out=is_all[:, i, :], in_max=vs_all[:, i, :],
                                    in_values=sarr[:, q, :])

            # reorder vs_all (s-descending) into biased-rank order by idx match
            nc.vector.tensor_copy(ibf[:, s4, :], ib_all[:, s4, :])
            nc.vector.tensor_copy(isf[:, s4, :], is_all[:, s4, :])
            eeng = nc.gpsimd if globals().get('_EQ_ON_POOL', 0) else nc.vector
            eq = scr.tile([128, tiles_ss, 8, 8], f32, tag="eq")
            eeng.tensor_tensor(
                out=eq[:],
                in0=ibf[:, s4, :].broadcast_to([128, tiles_ss, 8, 8]),
                in1=isf[:, s4, :][:, :, None, :].broadcast_to(
                    [128, tiles_ss, 8, 8]),
                op=Alu.is_equal,
            )
            g_sc = scr.tile([128, tiles_ss, 8, 8], f32, tag="g")
            eeng.tensor_tensor(
                out=g_sc[:], in0=eq[:],
                in1=vs_all[:, s4, :][:, :, None, :].broadcast_to(
                    [128, tiles_ss, 8, 8]),
                op=Alu.mult,
            )
            tsr = scr.tile([128, tiles_ss, 8], f32, tag="tsr")
            nc.vector.reduce_sum(out=tsr[:], in_=g_sc[:], axis=mybir.AxisListType.X)

            den = scr.tile([128, tiles_ss], f32, tag="den")
            nc.vector.reduce_sum(out=den[:], in_=vs_all[:, s4, :],
                                 axis=mybir.AxisListType.X)
            rec = scr.tile([128, tiles_ss], f32, tag="rec")
            nc.vector.reciprocal(rec[:], den[:])
            nc.vector.scalar_tensor_tensor(
                out=out_s_sb[:, s4, :], in0=tsr[:], scalar=ROUTE_SCALE,
                in1=rec[:].broadcast_to([128, tiles_ss, 8]),
                op0=Alu.mult, op1=Alu.mult,
            )
            if globals().get('_OUT_PER_ST', 0):
                od_s = out_s_d.rearrange("p (i k) -> p i k", k=8)
                od_i = out_i_d.rearrange("p (i k) -> p i k", k=8)
                nc.scalar.dma_start(od_s[:, s4, :], out_s_sb[:, s4, :])
                nc.scalar.dma_start(od_i[:, s4, :], ib_all[:, s4, :])

        schedule = SCHEDULE
        assert sum(schedule) == T_CORE

        def whole_pass():
            pos = 0
            n_early = globals().get('_EARLY_TILES', 12)
            hi_last = globals().get('_HI_LAST', 0)
            early_done = False
            for bi, tok_st in enumerate(schedule):
                if hi_last and bi >= len(schedule) - hi_last:
                    with tc.high_priority():
                        supertile(pos, tok_st)
                else:
                    supertile(pos, tok_st)
                pos += tok_st
                store_out = not skip_compute and not skip_topk and \
                    not globals().get('_OUT_PER_ST', 0)
                if store_out and not early_done and pos >= 128 * n_early:
                    # store the finished head tiles while the x stream still
                    # runs; only the short tail rides the final store pair
                    e = pos // 128
                    od_s = out_s_d.rearrange("p (i k) -> p i k", k=8)
                    od_i = out_i_d.rearrange("p (i k) -> p i k", k=8)
                    nc.scalar.dma_start(od_s[:, 0:e, :], out_s_sb[:, 0:e, :])
                    nc.scalar.dma_start(od_i[:, 0:e, :], ib_all[:, 0:e, :])
                    early_done = True
                    tail0 = e
            if not skip_compute and not skip_topk and not globals().get('_OUT_PER_ST', 0):
                t0 = tail0 if early_done else 0
                od_s = out_s_d.rearrange("p (i k) -> p i k", k=8)
                od_i = out_i_d.rearrange("p (i k) -> p i k", k=8)
                nc.scalar.dma_start(od_s[:, t0:, :], out_s_sb[:, t0:, :])
                nc.scalar.dma_start(od_i[:, t0:, :], ib_all[:, t0:, :])

        if reps == 1:
            whole_pass()
        else:
            with tc.For_i(0, reps, 1):
                whole_pass()


def build_nc(reps=1, skip_dma=False, skip_compute=False, n_terms=3, skip_topk=False):
    nc = bacc.Bacc("TRN2", target_bir_lowering=False, debug=False)
    xh_d = nc.dram_tensor("xh_d", [128, N_CH * T_CORE], f16, kind="ExternalInput")
    xl_d = nc.dram_tensor("xl_d", [128, N_CH * T_CORE], f8, kind="ExternalInput")
    w2_d = nc.dram_tensor("w2_d", [128, N_CH * 128], f16, kind="ExternalInput")
    w8_d = nc.dram_tensor("w8_d", [128, N_CH * E], f8, kind="ExternalInput")
    bias_d = nc.dram_tensor("bias_d", [128, E], f32, kind="ExternalInput")
    jab_d = nc.dram_tensor("jab_d", [128, E], f32, kind="ExternalInput")
    out_s_d = nc.dram_tensor("out_s_d", [128, N_TILES * 8], f32, kind="ExternalOutput")
    out_i_d = nc.dram_tensor("out_i_d", [128, N_TILES * 8], u32, kind="ExternalOutput")

    with tile.TileContext(nc) as tc:
        router_body(
            tc,
            (out_s_d.ap(), out_i_d.ap()),
            (xh_d.ap(), xl_d.ap(), w2_d.ap(), w8_d.ap(), bias_d.ap(), jab_d.ap()),
            reps=reps, skip_dma=skip_dma, skip_compute=skip_compute,
            n_terms=n_terms, skip_topk=skip_topk,
        )
    nc.compile()
    return nc


def pack_x_shard(xT, dtype):
    """[H, T_CORE] -> [128, N_CH*T_CORE] with each SCHEDULE block stored
    contiguously: out[p, N_CH*pos + c*tok_st + t] = xT[c*128 + p, pos + t]."""
    v = xT.reshape(N_CH, 128, T_CORE)
    blocks = []
    pos = 0
    for tok_st in SCHEDULE:
        blk = v[:, :, pos:pos + tok_st]            # [N_CH, 128, tok_st]
        blocks.append(blk.transpose(1, 0, 2).reshape(128, N_CH * tok_st))
        pos += tok_st
    return np.ascontiguousarray(np.concatenate(blocks, axis=1)).astype(dtype)


def pack_w2(wh, wl):
    """wh/wl [E, H] fp16 -> [128, N_CH*128] with wh in cols 0:64, wl in 64:128
    of each chunk: out[p, c*128 + e] = (wh if e < E else wl)[e % E, c*128 + p]."""
    vh = wh.T.reshape(N_CH, 128, E)
    vl = wl.T.reshape(N_CH, 128, E)
    v = np.concatenate([vh, vl], axis=2)          # [N_CH, 128, 128]
    return np.ascontiguousarray(v.transpose(1, 0, 2)).reshape(128, N_CH * 128)


def pack_w8(w):
    """w [E, H] f32 -> e4m3 [128, N_CH*E]: out[p, c*E + e] = w8[e, c*128+p]."""
    w8 = (w * 2.0 ** WB).astype(ml_dtypes.float8_e4m3)
    v = w8.T.reshape(N_CH, 128, E)
    return np.ascontiguousarray(v.transpose(1, 0, 2)).reshape(128, N_CH * E)


_NC_CACHE = {}


def host_pack(hidden_states, expert_bias, gate_w):
    x2 = np.asarray(hidden_states, dtype=np.float32).reshape(T_FULL, H)
    w = np.asarray(gate_w, dtype=np.float32)
    bias = np.asarray(expert_bias, dtype=np.float32)

    xh0 = x2.astype(np.float16)
    r = (x2 - xh0.astype(np.float32)) * float(2.0 ** XA)
    xh = (xh0.astype(np.float32) * float(2.0 ** XS)).astype(np.float16)
    ws = float(2.0 ** WS)
    wh = (w.astype(np.float16).astype(np.float32) * ws).astype(np.float16)
    wl = ((w - w.astype(np.float16).astype(np.float32)) * ws).astype(np.float16)

    w2_p = pack_w2(wh, wl)
    w8_p = pack_w8(w)
    bias_p = np.ascontiguousarray(np.broadcast_to(bias[None, :], (128, E)))
    jab = np.zeros((128, E), dtype=np.float32)
    jab[0:E, :] = np.eye(E) * CSCALE
    jab[E:2 * E, :] = np.eye(E) * CSCALE
    jab_p = jab

    in_maps = []
    for k in range(N_CORES):
        rows = slice(k * T_CORE, (k + 1) * T_CORE)
        in_maps.append({
            "xh_d": pack_x_shard(np.ascontiguousarray(xh[rows].T), np.float16),
            "xl_d": pack_x_shard(np.ascontiguousarray(r[rows].T.astype(np.float32)),
                                 ml_dtypes.float8_e4m3),
            "w2_d": w2_p,
            "w8_d": w8_p,
            "bias_d": bias_p,
            "jab_d": jab_p,
        })
    return in_maps


def kernel(hidden_states, expert_bias, gate_w):
    in_maps = host_pack(hidden_states, expert_bias, gate_w)

    if "nc" not in _NC_CACHE:
        _NC_CACHE["nc"] = build_nc()
    nc = _NC_CACHE["nc"]

    res = bass_utils.run_bass_kernel_spmd(nc, in_maps, core_ids=list(range(N_CORES)))

    scores = np.empty((T_FULL, TOPK), dtype=np.float32)
    idx = np.empty((T_FULL, TOPK), dtype=np.int32)
    for k in range(N_CORES):
        o = res.results[k]
        s = o["out_s_d"].reshape(128, N_TILES, TOPK).transpose(1, 0, 2).reshape(T_CORE, TOPK)
        i = o["out_i_d"].view(np.int32).reshape(128, N_TILES, TOPK).transpose(1, 0, 2).reshape(T_CORE, TOPK)
        scores[k * T_CORE:(k + 1) * T_CORE] = s
        idx[k * T_CORE:(k + 1) * T_CORE] = i
    return scores, idx
